# revision 33
# baseline (speedup 1.0000x reference)
"""Trainium2 Bass kernel for nn_MultiHeadAttention_42125039239620.

Semantics (faithful to reference.py):
  qh/kh/vh = per-head projections of q,k,v            [B,H,S,hd]
  scores   = qh @ kh^T / 8; masked rows/cols -> 0; causal strict-upper -> -inf
  attn     = softmax(scores); O = attn @ vh           [B,H,S,hd]
  out      = RAW VIEW of O as [B,S,H*hd] (memory reinterpretation, no head
             transpose!) @ Wo.
  The raw view decomposes per head: out[b, 128h:128(h+1), :] =
      O[b,h].reshape(128, 16*hd) @ Wo[0]
  so each (b, h) owns 128 exclusive output rows -> the 8-core unshard is a
  pure concatenation (no inter-core reduction).

Sharding: core c -> batch c//4, heads 4*(c%4) .. 4*(c%4)+3.

Device pipeline per core (bf16 matmuls, fp32 PSUM accumulate):
  - host feeds qT ( (q*keep/8)^T ), kT ( (k*keep)^T ), vT (v^T) in bf16; the
    pad mask is folded into q/k so masked score entries become exactly 0
    (exp(0)=1, matching the reference's where(pad, 0, scores)).
  - warmup matmuls at t=0 get the PE HAM clock-gate to 8/8 before real work.
  - projections produce qh^T/kh^T [64,S] per head (2 heads packed per matmul)
    and vh [t,hd] with an interleaved ones column (PV matmul then yields
    softmax denominators for free as psum row 64).
  - scores^T chunks [128t, 512s] per head pair: two K=64 matmuls at
    tile_position (0,0)/(64,0) -> they run CONCURRENTLY (row tiling);
    exp on ScalarE over the full [128, 1024] psum (scores are bounded);
    causal via triangular 0/1 mask multiplies (GpSimd) on diagonal chunks.
  - normalization: recip of sums (psum row 64) broadcast across partitions
    via GpSimd, fused into the PSUM->SBUF copy of O^T written in NATURAL
    layout, duplicated to partitions 0-63 and 64-127.
  - Wo stage: out[r, n] = sum_c sum_e O^T[e, 16r+c] Wo[64c+e, n]; per c a
    K=64 matmul with stride-16 lhsT; even c from partitions 0-63 (T0),
    odd c from partitions 64-127 (T8) -> concurrent pairs into two psum
    banks, combined with a GpSimd copy + DVE add.
"""

import sys

sys.path.insert(0, "/opt/trn_rl_repo")

import numpy as np
import ml_dtypes

import concourse.bacc as bacc
import concourse.tile as tile
from concourse.tile import add_dep_helper
import concourse.mybir as mybir
from concourse.bass_utils import run_bass_kernel_spmd

BF16 = ml_dtypes.bfloat16
FP32 = mybir.dt.float32
BF = mybir.dt.bfloat16

B, S, D = 2, 2048, 1024
H, HD = 16, 64
NC = 8          # cores
HL = 4          # heads per core
SC = 512        # s-chunk width (matmul free dim)
NJ = S // SC    # 4 s-chunks
TC = 128        # t-chunk width (psum partition dim)
NTC = S // TC   # 16 t-chunks
DC = D // 128   # 8 d-chunks

_PROGRAM = None


def _build_program():
    nc = bacc.Bacc("TRN2", target_bir_lowering=False, debug=False, num_devices=NC)

    qT = nc.dram_tensor("qT", [128, DC * S], BF, kind="ExternalInput")
    kT = nc.dram_tensor("kT", [128, DC * S], BF, kind="ExternalInput")
    vT = nc.dram_tensor("vT", [128, DC * S], BF, kind="ExternalInput")
    wq = nc.dram_tensor("wq", [128, DC * HL * HD], BF, kind="ExternalInput")
    wk = nc.dram_tensor("wk", [128, DC * HL * HD], BF, kind="ExternalInput")
    wv = nc.dram_tensor("wv", [128, DC * HL * HD], BF, kind="ExternalInput")
    wo = nc.dram_tensor("wo", [128, DC * D], BF, kind="ExternalInput")
    tri = nc.dram_tensor("tri", [128, 4 * 2 * SC], BF, kind="ExternalInput")
    out = nc.dram_tensor("out", [HL * TC, D], mybir.dt.float32, kind="ExternalOutput")

    with tile.TileContext(nc) as tc:
        with (
            tc.tile_pool(name="big", bufs=1) as big,
            tc.tile_pool(name="acts", bufs=1) as acts,
            tc.tile_pool(name="exp", bufs=7) as expp,
            tc.tile_pool(name="small", bufs=2) as small,
            tc.tile_pool(name="ostage", bufs=2) as ostage,
            tc.tile_pool(name="ps_a", bufs=2, space="PSUM") as ps_a,
            tc.tile_pool(name="ps_sc", bufs=2, space="PSUM") as ps_sc,
            tc.tile_pool(name="ps_o", bufs=2, space="PSUM") as ps_o,
        ):
            # ---- input DMA ---------------------------------------------------
            # qT/kT interleaved across the three queues (first exp needs both
            # full q and full k); then vT; small weights lead, wo trails.
            wq_sb = big.tile([128, DC, HL * HD], BF, tag="wq")
            nc.scalar.dma_start(wq_sb[:], wq[:])
            wk_sb = big.tile([128, DC, HL * HD], BF, tag="wk")
            nc.scalar.dma_start(wk_sb[:], wk[:])
            tri_sb = big.tile([128, 4, 2 * SC], BF, tag="tri")
            nc.gpsimd.dma_start(tri_sb[:], tri[:])
            wv_sb = big.tile([128, DC, HL * HD], BF, tag="wv")
            nc.scalar.dma_start(wv_sb[:], wv[:])

            qs = [nc.sync, nc.scalar, nc.gpsimd]
            qT_sb = big.tile([128, DC, S], BF, tag="qT")
            qTr = qT[:].rearrange("p (d s) -> p d s", s=S)
            nc.sync.dma_start(qT_sb[:, 0:4, :], qTr[:, 0:4, :])
            nc.sync.dma_start(qT_sb[:, 4:8, :], qTr[:, 4:8, :])
            kT_sb = big.tile([128, DC, S], BF, tag="kT")
            kTr = kT[:].rearrange("p (d s) -> p d s", s=S)
            nc.sync.dma_start(kT_sb[:, 0:4, :], kTr[:, 0:4, :])
            nc.sync.dma_start(kT_sb[:, 4:8, :], kTr[:, 4:8, :])
            vT_sb = big.tile([128, DC, S], BF, tag="vT")
            nc.sync.dma_start(vT_sb[:], vT[:].rearrange("p (d s) -> p d s", s=S))
            wo_sb = big.tile([128, DC, D], BF, tag="wo")
            nc.scalar.dma_start(wo_sb[:], wo[:])
            qT_c = [qT_sb[:, dc, :] for dc in range(DC)]
            kT_c = [kT_sb[:, dc, :] for dc in range(DC)]
            vT_c = [vT_sb[:, dc, :] for dc in range(DC)]

            # ---- projections -------------------------------------------------
            # qh^T / kh^T: [128 (= head pair, 2x64), S] bf16, per pair.
            qh_sb = acts.tile([128, 2, S], BF, tag="qh")
            kh_sb = acts.tile([128, 2, S], BF, tag="kh")
            vhp_sb = acts.tile([128, NTC, HL * 65], BF, tag="vhp")

            def qk_group(w_sb, src_c, dst, p, j, eng):
                ps = ps_a.tile([128, SC], FP32, tag="pa", name="psqk")
                for dc in range(DC):
                    nc.tensor.matmul(
                        ps[:],
                        w_sb[:, dc, 128 * p : 128 * (p + 1)],
                        src_c[dc][:, SC * j : SC * (j + 1)],
                        start=(dc == 0),
                        stop=(dc == DC - 1),
                    )
                eng.tensor_copy(dst[:, p, SC * j : SC * (j + 1)], ps[:])

            def vh_group(t):
                ps = ps_a.tile([128, SC], FP32, tag="pa", name="psv")
                for dc in range(DC):
                    nc.tensor.matmul(
                        ps[:, 0 : HL * HD],
                        vT_c[dc][:, TC * t : TC * (t + 1)],
                        wv_sb[:, dc, :],
                        start=(dc == 0),
                        stop=(dc == DC - 1),
                    )
                nc.vector.tensor_copy(
                    vhp_sb[:, t, :].rearrange("p (h w) -> p h w", w=65)[:, :, 0:64],
                    ps[:, 0 : HL * HD].rearrange("p (h w) -> p h w", w=64),
                )
                nc.gpsimd.memset(
                    vhp_sb[:, t, :].rearrange("p (h w) -> p h w", w=65)[:, :, 64:65],
                    1.0,
                )

            with nc.named_scope("proj_p0"):
                for j in range(NJ):
                    qk_group(wq_sb, qT_c, qh_sb, 0, j, nc.vector)
                for j in range(NJ):
                    qk_group(wk_sb, kT_c, kh_sb, 0, j, nc.vector)

            # p1 projections / Wo units are emitted as rationed fillers inside
            # the attention loops so they never starve the exp stream.
            fillers = []
            for j in range(NJ):
                fillers.append(
                    lambda j=j: qk_group(wq_sb, qT_c, qh_sb, 1, j, nc.vector)
                )
            for j in range(NJ):
                fillers.append(
                    lambda j=j: qk_group(wk_sb, kT_c, kh_sb, 1, j, nc.vector)
                )

            # ---- attention + Wo ---------------------------------------------
            oh_sb = acts.tile([128, HL, S], BF, tag="oh")  # O^T natural, dup'd

            def attention_pair(p):
                for j in range(NJ):  # noqa: B023
                    ntc = 4 * (j + 1)  # causal: t-chunks 0..ntc-1
                    vh_todo = []
                    if p == 0:
                        if j == 0:
                            for t in range(4):
                                vh_group(t)
                        if j < 3:
                            vh_todo = list(range(4 * (j + 1), 4 * (j + 2)))
                    o_ps = [
                        ps_o.tile([65, SC], FP32, tag="o", name=f"o{p}{j}{par}")
                        for par in range(2)
                    ]
                    e_tiles = {}

                    def pv(t):
                        e_prev, lo_prev = e_tiles.pop(t)
                        for par in range(2):
                            hl = 2 * p + par
                            mm = nc.tensor.matmul(
                                o_ps[par][:, lo_prev:],
                                vhp_sb[:, t, 65 * hl : 65 * hl + 65],
                                e_prev[:, SC * par + lo_prev : SC * (par + 1)],
                                start=(t == 0),
                                stop=(t == ntc - 1),
                                skip_group_check=True,
                            )
                            pass

                    prev_ts = []
                    for tb in range(0, ntc, 2):
                        ts = [t for t in (tb, tb + 1) if t < ntc]
                        for t in ts:  # scores pairs back-to-back in 64-mode
                            m = t - 4 * j
                            # causal truncation: cols < 128m are masked anyway.
                            # first use of each psum slot must be full-width
                            # (stale fp32 garbage would exp() to inf).
                            lo = 128 * m if m > 0 else 0
                            if p == 0 and j == 0 and t <= 1:
                                lo = 0
                            sc_ps = ps_sc.tile([128, 2 * SC], FP32, tag="sc")
                            for par in range(2):
                                off = 64 * par
                                nc.tensor.matmul(
                                    sc_ps[:, SC * par + lo : SC * (par + 1)],
                                    kh_sb[off : off + 64, p, TC * t : TC * (t + 1)],
                                    qh_sb[off : off + 64, p, SC * j + lo : SC * (j + 1)],
                                    start=True,
                                    stop=True,
                                    skip_group_check=True,
                                )
                            e_sb = expp.tile([128, 2 * SC], BF, tag="e")
                            if lo == 0:
                                nc.scalar.activation(
                                    e_sb[:], sc_ps[:],
                                    mybir.ActivationFunctionType.Exp,
                                )
                            else:
                                for par in range(2):
                                    nc.scalar.activation(
                                        e_sb[:, SC * par + lo : SC * (par + 1)],
                                        sc_ps[:, SC * par + lo : SC * (par + 1)],
                                        mybir.ActivationFunctionType.Exp,
                                    )
                            if m >= 0:
                                if lo == 0:
                                    nc.vector.tensor_mul(
                                        e_sb[:], e_sb[:], tri_sb[:, m, :]
                                    )
                                else:
                                    for par in range(2):
                                        nc.vector.tensor_mul(
                                            e_sb[:, SC * par + lo : SC * (par + 1)],
                                            e_sb[:, SC * par + lo : SC * (par + 1)],
                                            tri_sb[:, m, SC * par + lo : SC * (par + 1)],
                                        )
                            e_tiles[t] = (e_sb, 128 * m if m > 0 else 0)
                        for t in prev_ts:
                            pv(t)
                        if vh_todo:
                            vh_group(vh_todo.pop(0))
                        elif fillers:
                            fillers.pop(0)()
                        prev_ts = ts
                    for t in prev_ts:
                        pv(t)
                    while vh_todo:
                        vh_group(vh_todo.pop(0))

                    # normalize: recip(sums) broadcast over partitions, fused
                    # into the PSUM->SBUF copy; written twice (partitions 0-63
                    # and 64-127) so the Wo stage can pair even/odd c slices.
                    for par in range(2):
                        hl = 2 * p + par
                        sums_sb = small.tile([1, SC], FP32, tag="sums")
                        nc.vector.tensor_copy(sums_sb[:], o_ps[par][64:65, :])
                        rec_sb = small.tile([1, SC], FP32, tag="rec")
                        nc.vector.reciprocal_approx_fast(rec_sb[:], sums_sb[:])
                        bc_sb = small.tile([64, SC], FP32, tag="bc")
                        nc.gpsimd.partition_broadcast(
                            bc_sb[:], rec_sb[:], channels=64
                        )
                        nc.vector.tensor_mul(
                            oh_sb[0:64, hl, SC * j : SC * (j + 1)],
                            o_ps[par][0:64, :],
                            bc_sb[:],
                        )
                        nc.vector.tensor_copy(
                            oh_sb[64:128, hl, SC * j : SC * (j + 1)],
                            oh_sb[0:64, hl, SC * j : SC * (j + 1)],
                        )

            def wo_unit(hl, n, tail=False):
                ohp = oh_sb[:, hl, :].rearrange("p (m c) -> p c m", c=16)
                if tail:
                    f2 = ps_sc.tile([128, 2 * SC], FP32, tag="sc", name="fw2")
                    f_ev, f_od = f2[:, 0:SC], f2[:, SC : 2 * SC]
                else:
                    f_ev = ps_a.tile([128, SC], FP32, tag="pa", name="fwe")
                    f_od = ps_a.tile([128, SC], FP32, tag="pa", name="fwo")
                for cc in range(8):
                    nc.tensor.matmul(
                        f_ev[:],
                        ohp[0:64, 2 * cc, :],
                        wo_sb[0:64, cc, SC * n : SC * (n + 1)],
                        start=(cc == 0),
                        stop=(cc == 7),
                        skip_group_check=True,
                    )
                    nc.tensor.matmul(
                        f_od[:],
                        ohp[64:128, 2 * cc + 1, :],
                        wo_sb[64:128, cc, SC * n : SC * (n + 1)],
                        start=(cc == 0),
                        stop=(cc == 7),
                        skip_group_check=True,
                    )
                oc = ostage.tile([128, SC], FP32, tag="oc")
                if tail:
                    nc.scalar.activation(
                        oc[:], f_ev[:], mybir.ActivationFunctionType.Copy
                    )
                else:
                    nc.vector.tensor_copy(oc[:], f_ev[:])
                oc2 = ostage.tile([128, SC], FP32, tag="oc2")
                nc.vector.tensor_tensor(
                    oc2[:], f_od[:], oc[:], mybir.AluOpType.add
                )
                qs[(2 * hl + n) % 3].dma_start(
                    out[TC * hl : TC * (hl + 1), SC * n : SC * (n + 1)],
                    oc2[:],
                )

            def wo_stage(p):
                for par in range(2):
                    for n in range(2):
                        wo_unit(2 * p + par, n, tail=(p == 1))

            with nc.named_scope("att0"):
                attention_pair(0)
            with nc.named_scope("att1"):
                for par in range(2):
                    for n in range(2):
                        fillers.append(
                            lambda par=par, n=n: wo_unit(par, n)
                        )
                attention_pair(1)
            with nc.named_scope("wo1"):
                wo_stage(1)

    nc.compile()
    return nc


def _prep_inputs(q, k, v, Wq, Wk, Wv, Wo, mask):
    q = np.asarray(q, np.float32)
    k = np.asarray(k, np.float32)
    v = np.asarray(v, np.float32)
    Wq = np.asarray(Wq, np.float32)
    Wk = np.asarray(Wk, np.float32)
    Wv = np.asarray(Wv, np.float32)
    Wo = np.asarray(Wo, np.float32)
    mask = np.asarray(mask)

    keep = 1.0 - mask.astype(np.float32)  # [B, S]

    def chunk_major(xT):  # [D, S] -> [128, DC*S] partition-major
        return np.ascontiguousarray(
            xT.reshape(DC, 128, S).transpose(1, 0, 2).reshape(128, DC * S)
        )

    qTs, kTs, vTs = [], [], []
    for b in range(B):
        qTs.append(
            chunk_major(
                np.ascontiguousarray((q[b] * keep[b][:, None] * 0.125).T).astype(BF16)
            )
        )
        kTs.append(
            chunk_major(np.ascontiguousarray((k[b] * keep[b][:, None]).T).astype(BF16))
        )
        vTs.append(chunk_major(np.ascontiguousarray(v[b].T).astype(BF16)))

    def part_major(w):  # [D, N] -> [128, DC*N] with w[128c+p, n] at [p, c*N+n]
        n = w.shape[1]
        return np.ascontiguousarray(
            w.reshape(DC, 128, n).transpose(1, 0, 2).reshape(128, DC * n)
        )

    wqs, wks, wvs = [], [], []
    for g in range(4):
        hs = slice(4 * g, 4 * g + 4)
        wqs.append(
            part_major(np.transpose(Wq[0, hs], (1, 0, 2)).reshape(D, HL * HD).astype(BF16))
        )
        wks.append(
            part_major(np.transpose(Wk[0, hs], (1, 0, 2)).reshape(D, HL * HD).astype(BF16))
        )
        wvs.append(
            part_major(np.transpose(Wv[0, hs], (1, 0, 2)).reshape(D, HL * HD).astype(BF16))
        )
    wo_bf = part_major(Wo[0].astype(BF16))

    t_idx = np.arange(TC)[:, None]
    s_idx = np.arange(SC)[None, :]
    tri1 = np.stack([(128 * m + t_idx <= s_idx) for m in range(4)])  # [4,128,512]
    tri = np.ascontiguousarray(
        np.concatenate([tri1, tri1], axis=2)
        .astype(np.float32)
        .astype(BF16)
        .transpose(1, 0, 2)
        .reshape(128, 4 * 2 * SC)
    )

    in_maps = []
    for c in range(NC):
        b, g = c // 4, c % 4
        in_maps.append(
            {
                "qT": qTs[b],
                "kT": kTs[b],
                "vT": vTs[b],
                "wq": wqs[g],
                "wk": wks[g],
                "wv": wvs[g],
                "wo": wo_bf,
                "tri": tri,
            }
        )
    return in_maps


def _run(in_maps, trace=False):
    global _PROGRAM
    if _PROGRAM is None:
        _PROGRAM = _build_program()
    return run_bass_kernel_spmd(_PROGRAM, in_maps, list(range(NC)), trace=trace)


def kernel(q, k, v, Wq, Wk, Wv, Wo, mask, _trace=False):
    in_maps = _prep_inputs(q, k, v, Wq, Wk, Wv, Wo, mask)
    res = _run(in_maps, trace=_trace)
    final = np.zeros((B, S, D), np.float32)
    for c in range(NC):
        b, g = c // 4, c % 4
        final[b, 512 * g : 512 * (g + 1), :] = res.results[c]["out"]
    if _trace:
        kernel._last_exec_time_ns = res.exec_time_ns
        kernel._last_trace = res.instructions_and_trace
        kernel._last_profile_json = res.profile_json
        kernel._last_result = res
    return final


# revision 34
# speedup vs baseline: 1.0287x; 1.0287x over previous
"""Trainium2 Bass kernel for nn_MultiHeadAttention_42125039239620.

Semantics (faithful to reference.py):
  qh/kh/vh = per-head projections of q,k,v            [B,H,S,hd]
  scores   = qh @ kh^T / 8; masked rows/cols -> 0; causal strict-upper -> -inf
  attn     = softmax(scores); O = attn @ vh           [B,H,S,hd]
  out      = RAW VIEW of O as [B,S,H*hd] (memory reinterpretation, no head
             transpose!) @ Wo.
  The raw view decomposes per head: out[b, 128h:128(h+1), :] =
      O[b,h].reshape(128, 16*hd) @ Wo[0]
  so each (b, h) owns 128 exclusive output rows -> the 8-core unshard is a
  pure concatenation (no inter-core reduction).

Sharding: core c -> batch c//4, heads 4*(c%4) .. 4*(c%4)+3.

Device pipeline per core (bf16 matmuls, fp32 PSUM accumulate):
  - host feeds qT ( (q*keep/8)^T ), kT ( (k*keep)^T ), vT (v^T) in bf16; the
    pad mask is folded into q/k so masked score entries become exactly 0
    (exp(0)=1, matching the reference's where(pad, 0, scores)).
  - warmup matmuls at t=0 get the PE HAM clock-gate to 8/8 before real work.
  - projections produce qh^T/kh^T [64,S] per head (2 heads packed per matmul)
    and vh [t,hd] with an interleaved ones column (PV matmul then yields
    softmax denominators for free as psum row 64).
  - scores^T chunks [128t, 512s] per head pair: two K=64 matmuls at
    tile_position (0,0)/(64,0) -> they run CONCURRENTLY (row tiling);
    exp on ScalarE over the full [128, 1024] psum (scores are bounded);
    causal via triangular 0/1 mask multiplies (GpSimd) on diagonal chunks.
  - normalization: recip of sums (psum row 64) broadcast across partitions
    via GpSimd, fused into the PSUM->SBUF copy of O^T written in NATURAL
    layout, duplicated to partitions 0-63 and 64-127.
  - Wo stage: out[r, n] = sum_c sum_e O^T[e, 16r+c] Wo[64c+e, n]; per c a
    K=64 matmul with stride-16 lhsT; even c from partitions 0-63 (T0),
    odd c from partitions 64-127 (T8) -> concurrent pairs into two psum
    banks, combined with a GpSimd copy + DVE add.
"""

import sys

sys.path.insert(0, "/opt/trn_rl_repo")

import numpy as np
import ml_dtypes

import concourse.bacc as bacc
import concourse.tile as tile
from concourse.tile import add_dep_helper
import concourse.mybir as mybir
from concourse.bass_utils import run_bass_kernel_spmd

BF16 = ml_dtypes.bfloat16
FP32 = mybir.dt.float32
BF = mybir.dt.bfloat16

B, S, D = 2, 2048, 1024
H, HD = 16, 64
NC = 8          # cores
HL = 4          # heads per core
SC = 512        # s-chunk width (matmul free dim)
NJ = S // SC    # 4 s-chunks
TC = 128        # t-chunk width (psum partition dim)
NTC = S // TC   # 16 t-chunks
DC = D // 128   # 8 d-chunks

_PROGRAM = None


def _build_program():
    nc = bacc.Bacc("TRN2", target_bir_lowering=False, debug=False, num_devices=NC)

    qT = nc.dram_tensor("qT", [128, DC * S], BF, kind="ExternalInput")
    kT = nc.dram_tensor("kT", [128, DC * S], BF, kind="ExternalInput")
    vT = nc.dram_tensor("vT", [128, DC * S], BF, kind="ExternalInput")
    wq = nc.dram_tensor("wq", [128, DC * HL * HD], BF, kind="ExternalInput")
    wk = nc.dram_tensor("wk", [128, DC * HL * HD], BF, kind="ExternalInput")
    wv = nc.dram_tensor("wv", [128, DC * HL * HD], BF, kind="ExternalInput")
    wo = nc.dram_tensor("wo", [128, DC * D], BF, kind="ExternalInput")
    tri = nc.dram_tensor("tri", [128, 4 * 2 * SC], BF, kind="ExternalInput")
    out = nc.dram_tensor("out", [HL * TC, D], mybir.dt.float32, kind="ExternalOutput")

    with tile.TileContext(nc) as tc:
        with (
            tc.tile_pool(name="big", bufs=1) as big,
            tc.tile_pool(name="acts", bufs=1) as acts,
            tc.tile_pool(name="exp", bufs=7) as expp,
            tc.tile_pool(name="small", bufs=2) as small,
            tc.tile_pool(name="ostage", bufs=2) as ostage,
            tc.tile_pool(name="ps_a", bufs=2, space="PSUM") as ps_a,
            tc.tile_pool(name="ps_sc", bufs=2, space="PSUM") as ps_sc,
            tc.tile_pool(name="ps_o", bufs=2, space="PSUM") as ps_o,
        ):
            # ---- input DMA ---------------------------------------------------
            # qT/kT interleaved across the three queues (first exp needs both
            # full q and full k); then vT; small weights lead, wo trails.
            wq_sb = big.tile([128, DC, HL * HD], BF, tag="wq")
            nc.scalar.dma_start(wq_sb[:], wq[:])
            wk_sb = big.tile([128, DC, HL * HD], BF, tag="wk")
            nc.scalar.dma_start(wk_sb[:], wk[:])
            tri_sb = big.tile([128, 4, 2 * SC], BF, tag="tri")
            nc.gpsimd.dma_start(tri_sb[:], tri[:])
            wv_sb = big.tile([128, DC, HL * HD], BF, tag="wv")
            nc.scalar.dma_start(wv_sb[:], wv[:])

            qs = [nc.sync, nc.scalar, nc.gpsimd]
            qT_sb = big.tile([128, DC, S], BF, tag="qT")
            nc.sync.dma_start(qT_sb[:], qT[:].rearrange("p (d s) -> p d s", s=S))
            kT_sb = big.tile([128, DC, S], BF, tag="kT")
            nc.sync.dma_start(kT_sb[:], kT[:].rearrange("p (d s) -> p d s", s=S))
            vT_sb = big.tile([128, DC, S], BF, tag="vT")
            nc.sync.dma_start(vT_sb[:], vT[:].rearrange("p (d s) -> p d s", s=S))
            wo_sb = big.tile([128, DC, D], BF, tag="wo")
            nc.scalar.dma_start(wo_sb[:], wo[:])
            qT_c = [qT_sb[:, dc, :] for dc in range(DC)]
            kT_c = [kT_sb[:, dc, :] for dc in range(DC)]
            vT_c = [vT_sb[:, dc, :] for dc in range(DC)]

            # ---- projections -------------------------------------------------
            # qh^T / kh^T: [128 (= head pair, 2x64), S] bf16, per pair.
            qh_sb = acts.tile([128, 2, S], BF, tag="qh")
            kh_sb = acts.tile([128, 2, S], BF, tag="kh")
            vhp_sb = acts.tile([128, NTC, HL * 65], BF, tag="vhp")

            def qk_group(w_sb, src_c, dst, p, j, eng):
                ps = ps_a.tile([128, SC], FP32, tag="pa", name="psqk")
                for dc in range(DC):
                    nc.tensor.matmul(
                        ps[:],
                        w_sb[:, dc, 128 * p : 128 * (p + 1)],
                        src_c[dc][:, SC * j : SC * (j + 1)],
                        start=(dc == 0),
                        stop=(dc == DC - 1),
                    )
                eng.tensor_copy(dst[:, p, SC * j : SC * (j + 1)], ps[:])

            def vh_group(t):
                ps = ps_a.tile([128, SC], FP32, tag="pa", name="psv")
                for dc in range(DC):
                    nc.tensor.matmul(
                        ps[:, 0 : HL * HD],
                        vT_c[dc][:, TC * t : TC * (t + 1)],
                        wv_sb[:, dc, :],
                        start=(dc == 0),
                        stop=(dc == DC - 1),
                    )
                nc.vector.tensor_copy(
                    vhp_sb[:, t, :].rearrange("p (h w) -> p h w", w=65)[:, :, 0:64],
                    ps[:, 0 : HL * HD].rearrange("p (h w) -> p h w", w=64),
                )
                nc.gpsimd.memset(
                    vhp_sb[:, t, :].rearrange("p (h w) -> p h w", w=65)[:, :, 64:65],
                    1.0,
                )

            with nc.named_scope("proj_p0"):
                for j in range(NJ):
                    qk_group(wq_sb, qT_c, qh_sb, 0, j, nc.vector)
                for j in range(NJ):
                    qk_group(wk_sb, kT_c, kh_sb, 0, j, nc.vector)

            # p1 projections / Wo units are emitted as rationed fillers inside
            # the attention loops so they never starve the exp stream.
            fillers = []
            for j in range(NJ):
                fillers.append(
                    lambda j=j: qk_group(wq_sb, qT_c, qh_sb, 1, j, nc.vector)
                )
            for j in range(NJ):
                fillers.append(
                    lambda j=j: qk_group(wk_sb, kT_c, kh_sb, 1, j, nc.vector)
                )

            # ---- attention + Wo ---------------------------------------------
            oh_sb = acts.tile([128, HL, S], BF, tag="oh")  # O^T natural, dup'd

            def attention_pair(p):
                for j in range(NJ):  # noqa: B023
                    ntc = 4 * (j + 1)  # causal: t-chunks 0..ntc-1
                    vh_todo = []
                    if p == 0:
                        if j == 0:
                            for t in range(4):
                                vh_group(t)
                        if j < 3:
                            vh_todo = list(range(4 * (j + 1), 4 * (j + 2)))
                    o_ps = [
                        ps_o.tile([65, SC], FP32, tag="o", name=f"o{p}{j}{par}")
                        for par in range(2)
                    ]
                    e_tiles = {}

                    def pv(t):
                        e_prev, lo_prev = e_tiles.pop(t)
                        for par in range(2):
                            hl = 2 * p + par
                            mm = nc.tensor.matmul(
                                o_ps[par][:, lo_prev:],
                                vhp_sb[:, t, 65 * hl : 65 * hl + 65],
                                e_prev[:, SC * par + lo_prev : SC * (par + 1)],
                                start=(t == 0),
                                stop=(t == ntc - 1),
                                skip_group_check=True,
                            )
                            pass

                    prev_ts = []
                    for tb in range(0, ntc, 2):
                        ts = [t for t in (tb, tb + 1) if t < ntc]
                        for t in ts:  # scores pairs back-to-back in 64-mode
                            m = t - 4 * j
                            # causal truncation: cols < 128m are masked anyway.
                            # first use of each psum slot must be full-width
                            # (stale fp32 garbage would exp() to inf).
                            lo = 128 * m if m > 0 else 0
                            if p == 0 and j == 0 and t <= 1:
                                lo = 0
                            sc_ps = ps_sc.tile([128, 2 * SC], FP32, tag="sc")
                            for par in range(2):
                                off = 64 * par
                                nc.tensor.matmul(
                                    sc_ps[:, SC * par + lo : SC * (par + 1)],
                                    kh_sb[off : off + 64, p, TC * t : TC * (t + 1)],
                                    qh_sb[off : off + 64, p, SC * j + lo : SC * (j + 1)],
                                    start=True,
                                    stop=True,
                                    skip_group_check=True,
                                )
                            e_sb = expp.tile([128, 2 * SC], BF, tag="e")
                            if lo == 0:
                                nc.scalar.activation(
                                    e_sb[:], sc_ps[:],
                                    mybir.ActivationFunctionType.Exp,
                                )
                            else:
                                for par in range(2):
                                    nc.scalar.activation(
                                        e_sb[:, SC * par + lo : SC * (par + 1)],
                                        sc_ps[:, SC * par + lo : SC * (par + 1)],
                                        mybir.ActivationFunctionType.Exp,
                                    )
                            if m >= 0:
                                if lo == 0:
                                    nc.vector.tensor_mul(
                                        e_sb[:], e_sb[:], tri_sb[:, m, :]
                                    )
                                else:
                                    for par in range(2):
                                        nc.vector.tensor_mul(
                                            e_sb[:, SC * par + lo : SC * (par + 1)],
                                            e_sb[:, SC * par + lo : SC * (par + 1)],
                                            tri_sb[:, m, SC * par + lo : SC * (par + 1)],
                                        )
                            e_tiles[t] = (e_sb, 128 * m if m > 0 else 0)
                        for t in prev_ts:
                            pv(t)
                        if vh_todo:
                            vh_group(vh_todo.pop(0))
                        elif fillers:
                            fillers.pop(0)()
                        prev_ts = ts
                    for t in prev_ts:
                        pv(t)
                    while vh_todo:
                        vh_group(vh_todo.pop(0))

                    # normalize: recip(sums) broadcast over partitions, fused
                    # into the PSUM->SBUF copy; written twice (partitions 0-63
                    # and 64-127) so the Wo stage can pair even/odd c slices.
                    for par in range(2):
                        hl = 2 * p + par
                        sums_sb = small.tile([1, SC], FP32, tag="sums")
                        nc.vector.tensor_copy(sums_sb[:], o_ps[par][64:65, :])
                        rec_sb = small.tile([1, SC], FP32, tag="rec")
                        nc.vector.reciprocal_approx_fast(rec_sb[:], sums_sb[:])
                        bc_sb = small.tile([64, SC], FP32, tag="bc")
                        nc.gpsimd.partition_broadcast(
                            bc_sb[:], rec_sb[:], channels=64
                        )
                        nc.vector.tensor_mul(
                            oh_sb[0:64, hl, SC * j : SC * (j + 1)],
                            o_ps[par][0:64, :],
                            bc_sb[:],
                        )
                        nc.vector.tensor_copy(
                            oh_sb[64:128, hl, SC * j : SC * (j + 1)],
                            oh_sb[0:64, hl, SC * j : SC * (j + 1)],
                        )

            def wo_unit(hl, n, tail=False):
                ohp = oh_sb[:, hl, :].rearrange("p (m c) -> p c m", c=16)
                if tail:
                    f2 = ps_sc.tile([128, 2 * SC], FP32, tag="sc", name="fw2")
                    f_ev, f_od = f2[:, 0:SC], f2[:, SC : 2 * SC]
                else:
                    f_ev = ps_a.tile([128, SC], FP32, tag="pa", name="fwe")
                    f_od = ps_a.tile([128, SC], FP32, tag="pa", name="fwo")
                for cc in range(8):
                    nc.tensor.matmul(
                        f_ev[:],
                        ohp[0:64, 2 * cc, :],
                        wo_sb[0:64, cc, SC * n : SC * (n + 1)],
                        start=(cc == 0),
                        stop=(cc == 7),
                        skip_group_check=True,
                    )
                    nc.tensor.matmul(
                        f_od[:],
                        ohp[64:128, 2 * cc + 1, :],
                        wo_sb[64:128, cc, SC * n : SC * (n + 1)],
                        start=(cc == 0),
                        stop=(cc == 7),
                        skip_group_check=True,
                    )
                oc = ostage.tile([128, SC], FP32, tag="oc")
                if tail:
                    nc.scalar.activation(
                        oc[:], f_ev[:], mybir.ActivationFunctionType.Copy
                    )
                else:
                    nc.vector.tensor_copy(oc[:], f_ev[:])
                oc2 = ostage.tile([128, SC], FP32, tag="oc2")
                nc.vector.tensor_tensor(
                    oc2[:], f_od[:], oc[:], mybir.AluOpType.add
                )
                qs[(2 * hl + n) % 3].dma_start(
                    out[TC * hl : TC * (hl + 1), SC * n : SC * (n + 1)],
                    oc2[:],
                )

            def wo_stage(p):
                for par in range(2):
                    for n in range(2):
                        wo_unit(2 * p + par, n, tail=(p == 1))

            with nc.named_scope("att0"):
                attention_pair(0)
            with nc.named_scope("att1"):
                for par in range(2):
                    for n in range(2):
                        fillers.append(
                            lambda par=par, n=n: wo_unit(par, n)
                        )
                attention_pair(1)
            with nc.named_scope("wo1"):
                wo_stage(1)

    nc.compile()
    return nc


def _prep_inputs(q, k, v, Wq, Wk, Wv, Wo, mask):
    q = np.asarray(q, np.float32)
    k = np.asarray(k, np.float32)
    v = np.asarray(v, np.float32)
    Wq = np.asarray(Wq, np.float32)
    Wk = np.asarray(Wk, np.float32)
    Wv = np.asarray(Wv, np.float32)
    Wo = np.asarray(Wo, np.float32)
    mask = np.asarray(mask)

    keep = 1.0 - mask.astype(np.float32)  # [B, S]

    def chunk_major(xT):  # [D, S] -> [128, DC*S] partition-major
        return np.ascontiguousarray(
            xT.reshape(DC, 128, S).transpose(1, 0, 2).reshape(128, DC * S)
        )

    qTs, kTs, vTs = [], [], []
    for b in range(B):
        qTs.append(
            chunk_major(
                np.ascontiguousarray((q[b] * keep[b][:, None] * 0.125).T).astype(BF16)
            )
        )
        kTs.append(
            chunk_major(np.ascontiguousarray((k[b] * keep[b][:, None]).T).astype(BF16))
        )
        vTs.append(chunk_major(np.ascontiguousarray(v[b].T).astype(BF16)))

    def part_major(w):  # [D, N] -> [128, DC*N] with w[128c+p, n] at [p, c*N+n]
        n = w.shape[1]
        return np.ascontiguousarray(
            w.reshape(DC, 128, n).transpose(1, 0, 2).reshape(128, DC * n)
        )

    wqs, wks, wvs = [], [], []
    for g in range(4):
        hs = slice(4 * g, 4 * g + 4)
        wqs.append(
            part_major(np.transpose(Wq[0, hs], (1, 0, 2)).reshape(D, HL * HD).astype(BF16))
        )
        wks.append(
            part_major(np.transpose(Wk[0, hs], (1, 0, 2)).reshape(D, HL * HD).astype(BF16))
        )
        wvs.append(
            part_major(np.transpose(Wv[0, hs], (1, 0, 2)).reshape(D, HL * HD).astype(BF16))
        )
    wo_bf = part_major(Wo[0].astype(BF16))

    t_idx = np.arange(TC)[:, None]
    s_idx = np.arange(SC)[None, :]
    tri1 = np.stack([(128 * m + t_idx <= s_idx) for m in range(4)])  # [4,128,512]
    tri = np.ascontiguousarray(
        np.concatenate([tri1, tri1], axis=2)
        .astype(np.float32)
        .astype(BF16)
        .transpose(1, 0, 2)
        .reshape(128, 4 * 2 * SC)
    )

    in_maps = []
    for c in range(NC):
        b, g = c // 4, c % 4
        in_maps.append(
            {
                "qT": qTs[b],
                "kT": kTs[b],
                "vT": vTs[b],
                "wq": wqs[g],
                "wk": wks[g],
                "wv": wvs[g],
                "wo": wo_bf,
                "tri": tri,
            }
        )
    return in_maps


def _run(in_maps, trace=False):
    global _PROGRAM
    if _PROGRAM is None:
        _PROGRAM = _build_program()
    return run_bass_kernel_spmd(_PROGRAM, in_maps, list(range(NC)), trace=trace)


def kernel(q, k, v, Wq, Wk, Wv, Wo, mask, _trace=False):
    in_maps = _prep_inputs(q, k, v, Wq, Wk, Wv, Wo, mask)
    res = _run(in_maps, trace=_trace)
    final = np.zeros((B, S, D), np.float32)
    for c in range(NC):
        b, g = c // 4, c % 4
        final[b, 512 * g : 512 * (g + 1), :] = res.results[c]["out"]
    if _trace:
        kernel._last_exec_time_ns = res.exec_time_ns
        kernel._last_trace = res.instructions_and_trace
        kernel._last_profile_json = res.profile_json
        kernel._last_result = res
    return final
